# revision 2
# baseline (speedup 1.0000x reference)
"""LokrLinear TRN2 kernel: out = x @ (W + 2*kron(A@B, O)) + b.

Sharding (8 cores, column-parallel per the hint):
  - Each core owns a 512-column slice of out_features: W[:, c*512:(c+1)*512]
    and the matching 2-column slice of (A@B) (since kron block size = 256).
  - x is replicated, passed pre-transposed as xT (in_f, rows) in bf16.
  - On device: fold W_eff = W_slice + 2*kron((A@B)[:, 2c:2c+2], O) into
    SBUF-resident bf16 weights, then one dense matmul
    out_slice.T = W_eff.T @ xT with fp32 PSUM accumulation; bias is added
    per-partition during PSUM eviction.
  - Host gathers the 8 (512, 16384) outputs, transposes, reshapes.
"""

import numpy as np
import ml_dtypes

P = 128
IN_F = 4096
OUT_F = 4096
ROWS = 4 * 4096            # 16384
N_CORES = 8
COLS = OUT_F // N_CORES    # 512 out_features per core
R = 16                     # LoKr rank
OB = 256                   # O block size (kron block)
JB = COLS // OB            # j-blocks per core = 2
SCALING = 2.0
NS = 512                   # rows per n-slice (one PSUM bank of fp32)
KO = IN_F // P             # 32 k-tiles
MT = COLS // P             # 4 m-tiles

_CACHE = {}


def build_nc(n_slices=ROWS // NS, debug=False):
    """Build the per-core Bass program. Identical on all cores (SPMD);
    core-specific data arrives via the input tensors."""
    import concourse.bass as bass  # noqa: F401
    import concourse.mybir as mybir
    import concourse.tile as tile
    from concourse import bacc

    f32 = mybir.dt.float32
    bf16 = mybir.dt.bfloat16
    Copy = mybir.ActivationFunctionType.Copy
    rows = n_slices * NS

    nc = bacc.Bacc("TRN2", target_bir_lowering=False, debug=debug)

    xt = nc.dram_tensor("xt", (IN_F, rows), bf16, kind="ExternalInput")
    wk = nc.dram_tensor("wk", (IN_F, COLS), bf16, kind="ExternalInput")
    bias = nc.dram_tensor("bias", (COLS,), f32, kind="ExternalInput")
    a_t = nc.dram_tensor("a_t", (R, R), f32, kind="ExternalInput")
    b_sl = nc.dram_tensor("b_sl", (R, JB), f32, kind="ExternalInput")
    o_mat = nc.dram_tensor("o_mat", (OB, OB), f32, kind="ExternalInput")
    out = nc.dram_tensor("out", (COLS, rows), f32, kind="ExternalOutput")
    wl_scr = nc.dram_tensor("wl_scr", (R, JB), f32, kind="Internal")

    with tile.TileContext(nc) as tc:
        with (
            tc.tile_pool(name="const", bufs=1) as cst,
            tc.tile_pool(name="weff", bufs=KO) as weff_pool,
            tc.tile_pool(name="wraw", bufs=3) as wraw_pool,
            tc.tile_pool(name="upd", bufs=2) as upd_pool,
            tc.tile_pool(name="xts", bufs=3) as xts_pool,
            tc.tile_pool(name="outp", bufs=4) as out_pool,
            tc.tile_pool(name="ps", bufs=7, space="PSUM") as ps_pool,
            tc.tile_pool(name="wlps", bufs=1, space="PSUM") as wlps_pool,
        ):
            # ---- w_left slice = 2 * (A @ B[:, 2c:2c+2]) --------------------
            a_sb = cst.tile((R, R), f32, name="a_sb")
            nc.sync.dma_start(out=a_sb[:], in_=a_t[:, :])
            b_sb = cst.tile((R, JB), f32, name="b_sb")
            nc.sync.dma_start(out=b_sb[:], in_=b_sl[:, :])
            wl_ps = wlps_pool.tile((R, JB), f32, name="wl_ps")
            nc.tensor.matmul(wl_ps[:], a_sb[:], b_sb[:], start=True, stop=True)
            wl_sb = cst.tile((R, JB), f32, name="wl_sb")
            nc.scalar.activation(wl_sb[:], wl_ps[:], Copy, scale=SCALING)
            nc.sync.dma_start(out=wl_scr[:, :], in_=wl_sb[:])
            # broadcast the R*JB scalars to every partition: (P, R*JB)
            wl_b = cst.tile((P, R * JB), f32, name="wl_b")
            src = wl_scr[:, :].rearrange("i j -> (i j)")[None, :].to_broadcast(
                [P, R * JB]
            )
            nc.sync.dma_start(out=wl_b[:], in_=src)

            # ---- constants -------------------------------------------------
            o_sb = cst.tile((P, OB // P, OB), f32, name="o_sb")
            nc.sync.dma_start(
                out=o_sb[:], in_=o_mat[:, :].rearrange("(ah p) b -> p ah b", p=P)
            )
            bias_sb = cst.tile((P, MT), f32, name="bias_sb")
            nc.sync.dma_start(
                out=bias_sb[:], in_=bias[:].rearrange("(m p) -> p m", p=P)
            )

            # ---- fold W_eff = W + 2*kron(w_left, O) (per 128-row k-tile) ---
            wk_r = wk[:, :].rearrange("(ko p) n -> p ko n", p=P)
            weff = []
            for kt in range(KO):
                i, ah = kt // 2, kt % 2
                wraw = wraw_pool.tile((P, COLS), bf16, name="wraw")
                nc.sync.dma_start(out=wraw[:], in_=wk_r[:, kt, :])
                upd = upd_pool.tile((P, COLS), f32, name="upd")
                for jj in range(JB):
                    nc.vector.tensor_scalar_mul(
                        out=upd[:, jj * OB : (jj + 1) * OB],
                        in0=o_sb[:, ah, :],
                        scalar1=wl_b[:, i * JB + jj : i * JB + jj + 1],
                    )
                wt = weff_pool.tile((P, COLS), bf16, name="wt")
                nc.vector.tensor_add(out=wt[:], in0=wraw[:], in1=upd[:])
                weff.append(wt)

            # ---- main matmul: out.T = W_eff.T @ xT + bias ------------------
            xt_r = xt[:, :].rearrange("(ko p) n -> p ko n", p=P)
            out_r = out[:, :].rearrange("(m p) n -> p m n", p=P)
            for ns in range(n_slices):
                xts = xts_pool.tile((P, KO, NS), bf16, name="xts")
                nc.sync.dma_start(
                    out=xts[:], in_=xt_r[:, :, ns * NS : (ns + 1) * NS]
                )
                for m in range(MT):
                    ps = ps_pool.tile((P, NS), f32, name="ps")
                    for kt in range(KO):
                        nc.tensor.matmul(
                            ps[:],
                            weff[kt][:, m * P : (m + 1) * P],
                            xts[:, kt, :],
                            start=(kt == 0),
                            stop=(kt == KO - 1),
                        )
                    ot = out_pool.tile((P, NS), f32, name="ot")
                    nc.scalar.activation(
                        ot[:],
                        ps[:],
                        mybir.ActivationFunctionType.Identity,
                        bias=bias_sb[:, m : m + 1],
                    )
                    nc.sync.dma_start(
                        out=out_r[:, m, ns * NS : (ns + 1) * NS], in_=ot[:]
                    )

    nc.compile()
    return nc


def _prep_inputs(x, base_kernel, base_bias, lora_A, lora_B, O):
    bf16 = ml_dtypes.bfloat16
    x2d = np.asarray(x, dtype=np.float32).reshape(ROWS, IN_F)
    xt = x2d.T.astype(bf16)  # (IN_F, ROWS) contiguous bf16
    a_t = np.ascontiguousarray(np.asarray(lora_A, np.float32).T)
    o_mat = np.ascontiguousarray(np.asarray(O, np.float32))
    in_maps = []
    for c in range(N_CORES):
        in_maps.append(
            {
                "xt": xt,
                "wk": np.asarray(
                    base_kernel[:, c * COLS : (c + 1) * COLS], np.float32
                ).astype(bf16),
                "bias": np.ascontiguousarray(
                    np.asarray(base_bias[c * COLS : (c + 1) * COLS], np.float32)
                ),
                "a_t": a_t,
                "b_sl": np.ascontiguousarray(
                    np.asarray(lora_B[:, c * JB : (c + 1) * JB], np.float32)
                ),
                "o_mat": o_mat,
            }
        )
    return in_maps


def kernel(x, base_kernel, base_bias, lora_A, lora_B, O, _trace=False):
    from concourse.bass_utils import run_bass_kernel_spmd

    if "nc" not in _CACHE:
        _CACHE["nc"] = build_nc()
    nc = _CACHE["nc"]
    in_maps = _prep_inputs(x, base_kernel, base_bias, lora_A, lora_B, O)
    res = run_bass_kernel_spmd(
        nc, in_maps, core_ids=list(range(N_CORES)), trace=_trace
    )
    _CACHE["last_results"] = res
    big = np.concatenate([r["out"] for r in res.results], axis=0)  # (OUT_F, ROWS)
    return np.ascontiguousarray(big.T).reshape(4, ROWS // 4, OUT_F)


# revision 8
# speedup vs baseline: 1.0163x; 1.0163x over previous
"""LokrLinear TRN2 kernel: out = x @ (W + 2*kron(A@B, O)) + b.

Sharding (8 cores, column-parallel per the hint):
  - Each core owns a 512-column slice of out_features: W[:, c*512:(c+1)*512]
    and the matching 2-column slice of (A@B) (since kron block size = 256).
  - x is replicated, passed pre-transposed as xT (in_f, rows) in bf16.
  - On device: fold W_eff = W_slice + 2*kron((A@B)[:, 2c:2c+2], O) into
    SBUF-resident bf16 weights, then one dense matmul
    out_slice.T = W_eff.T @ xT with fp32 PSUM accumulation; bias is added
    per-partition during PSUM eviction.
  - Host gathers the 8 (512, 16384) outputs, transposes, reshapes.
"""

import numpy as np
import ml_dtypes

P = 128
IN_F = 4096
OUT_F = 4096
ROWS = 4 * 4096            # 16384
N_CORES = 8
COLS = OUT_F // N_CORES    # 512 out_features per core
R = 16                     # LoKr rank
OB = 256                   # O block size (kron block)
JB = COLS // OB            # j-blocks per core = 2
SCALING = 2.0
NS = 512                   # rows per n-slice (one PSUM bank of fp32)
KO = IN_F // P             # 32 k-tiles
MT = COLS // P             # 4 m-tiles

_CACHE = {}


def build_nc(n_slices=ROWS // NS, debug=False):
    """Build the per-core Bass program. Identical on all cores (SPMD);
    core-specific data arrives via the input tensors."""
    import concourse.bass as bass  # noqa: F401
    import concourse.mybir as mybir
    import concourse.tile as tile
    from concourse import bacc

    f32 = mybir.dt.float32
    bf16 = mybir.dt.bfloat16
    rows = n_slices * NS

    nc = bacc.Bacc("TRN2", target_bir_lowering=False, debug=debug)

    xt = nc.dram_tensor("xt", (IN_F, rows), bf16, kind="ExternalInput")
    wk = nc.dram_tensor("wk", (IN_F, COLS), bf16, kind="ExternalInput")
    bias = nc.dram_tensor("bias", (COLS,), f32, kind="ExternalInput")
    a_t = nc.dram_tensor("a_t", (R, R), f32, kind="ExternalInput")
    b_sl = nc.dram_tensor("b_sl", (R, JB), f32, kind="ExternalInput")
    o_mat = nc.dram_tensor("o_mat", (OB, OB), bf16, kind="ExternalInput")
    out = nc.dram_tensor("out", (COLS, rows), f32, kind="ExternalOutput")
    wl_scr = nc.dram_tensor("wl_scr", (R, JB), f32, kind="Internal")

    with tile.TileContext(nc) as tc:
        with (
            tc.tile_pool(name="const", bufs=1) as cst,
            tc.tile_pool(name="weff", bufs=KO) as weff_pool,
            tc.tile_pool(name="wraw", bufs=3) as wraw_pool,
            tc.tile_pool(name="upd", bufs=2) as upd_pool,
            tc.tile_pool(name="xts", bufs=3) as xts_pool,
            tc.tile_pool(name="outp", bufs=4) as out_pool,
            tc.tile_pool(name="ps", bufs=7, space="PSUM") as ps_pool,
            tc.tile_pool(name="wlps", bufs=1, space="PSUM") as wlps_pool,
        ):
            # ---- w_left slice = 2 * (A @ B[:, 2c:2c+2]) --------------------
            a_sb = cst.tile((R, R), f32, name="a_sb")
            nc.sync.dma_start(out=a_sb[:], in_=a_t[:, :])
            b_sb = cst.tile((R, JB), f32, name="b_sb")
            nc.sync.dma_start(out=b_sb[:], in_=b_sl[:, :])
            wl_ps = wlps_pool.tile((R, JB), f32, name="wl_ps")
            nc.tensor.matmul(wl_ps[:], a_sb[:], b_sb[:], start=True, stop=True)
            wl_sb = cst.tile((R, JB), f32, name="wl_sb")
            nc.vector.tensor_copy(out=wl_sb[:], in_=wl_ps[:])
            nc.sync.dma_start(out=wl_scr[:, :], in_=wl_sb[:])
            # broadcast the R*JB scalars to every partition: (P, R*JB)
            wl_b = cst.tile((P, R * JB), f32, name="wl_b")
            src = wl_scr[:, :].rearrange("i j -> (i j)")[None, :].to_broadcast(
                [P, R * JB]
            )
            nc.sync.dma_start(out=wl_b[:], in_=src)

            # ---- constants -------------------------------------------------
            o_sb = cst.tile((P, OB // P, OB), bf16, name="o_sb")
            nc.sync.dma_start(
                out=o_sb[:], in_=o_mat[:, :].rearrange("(ah p) b -> p ah b", p=P)
            )
            bias_sb = cst.tile((P, MT), f32, name="bias_sb")
            nc.sync.dma_start(
                out=bias_sb[:], in_=bias[:].rearrange("(m p) -> p m", p=P)
            )

            # ---- fold W_eff = W + 2*kron(w_left, O) (per 128-row k-tile) ---
            wk_r = wk[:, :].rearrange("(ko p) n -> p ko n", p=P)
            weff = []
            for kt in range(KO):
                i, ah = kt // 2, kt % 2
                wraw = wraw_pool.tile((P, COLS), bf16, name="wraw")
                nc.sync.dma_start(out=wraw[:], in_=wk_r[:, kt, :])
                upd = upd_pool.tile((P, COLS), bf16, name="upd")
                for jj in range(JB):
                    nc.vector.tensor_scalar(
                        out=upd[:, jj * OB : (jj + 1) * OB],
                        in0=o_sb[:, ah, :],
                        scalar1=wl_b[:, i * JB + jj : i * JB + jj + 1],
                        scalar2=SCALING,
                        op0=mybir.AluOpType.mult,
                        op1=mybir.AluOpType.mult,
                    )
                wt = weff_pool.tile((P, COLS), bf16, name="wt")
                nc.vector.tensor_add(out=wt[:], in0=wraw[:], in1=upd[:])
                weff.append(wt)

            # ---- main matmul: out.T = W_eff.T @ xT + bias ------------------
            xt_r = xt[:, :].rearrange("(ko p) n -> p ko n", p=P)
            out_r = out[:, :].rearrange("(m p) n -> p m n", p=P)
            for ns in range(n_slices):
                xts = xts_pool.tile((P, KO, NS), bf16, name="xts")
                nc.sync.dma_start(
                    out=xts[:], in_=xt_r[:, :, ns * NS : (ns + 1) * NS]
                )
                for m in range(MT):
                    ps = ps_pool.tile((P, NS), f32, name="ps")
                    for kt in range(KO):
                        nc.tensor.matmul(
                            ps[:],
                            weff[kt][:, m * P : (m + 1) * P],
                            xts[:, kt, :],
                            start=(kt == 0),
                            stop=(kt == KO - 1),
                        )
                    ot = out_pool.tile((P, NS), f32, name="ot")
                    nc.scalar.activation(
                        ot[:],
                        ps[:],
                        mybir.ActivationFunctionType.Identity,
                        bias=bias_sb[:, m : m + 1],
                    )
                    nc.sync.dma_start(
                        out=out_r[:, m, ns * NS : (ns + 1) * NS], in_=ot[:]
                    )

    nc.compile()
    return nc


def _prep_inputs(x, base_kernel, base_bias, lora_A, lora_B, O):
    bf16 = ml_dtypes.bfloat16
    x2d = np.asarray(x, dtype=np.float32).reshape(ROWS, IN_F)
    xt = x2d.T.astype(bf16)  # (IN_F, ROWS) contiguous bf16
    a_t = np.ascontiguousarray(np.asarray(lora_A, np.float32).T)
    o_mat = np.asarray(O, np.float32).astype(bf16)
    in_maps = []
    for c in range(N_CORES):
        in_maps.append(
            {
                "xt": xt,
                "wk": np.asarray(
                    base_kernel[:, c * COLS : (c + 1) * COLS], np.float32
                ).astype(bf16),
                "bias": np.ascontiguousarray(
                    np.asarray(base_bias[c * COLS : (c + 1) * COLS], np.float32)
                ),
                "a_t": a_t,
                "b_sl": np.ascontiguousarray(
                    np.asarray(lora_B[:, c * JB : (c + 1) * JB], np.float32)
                ),
                "o_mat": o_mat,
            }
        )
    return in_maps


def kernel(x, base_kernel, base_bias, lora_A, lora_B, O, _trace=False):
    from concourse.bass_utils import run_bass_kernel_spmd

    if "nc" not in _CACHE:
        _CACHE["nc"] = build_nc()
    nc = _CACHE["nc"]
    in_maps = _prep_inputs(x, base_kernel, base_bias, lora_A, lora_B, O)
    res = run_bass_kernel_spmd(
        nc, in_maps, core_ids=list(range(N_CORES)), trace=_trace
    )
    _CACHE["last_results"] = res
    big = np.concatenate([r["out"] for r in res.results], axis=0)  # (OUT_F, ROWS)
    return np.ascontiguousarray(big.T).reshape(4, ROWS // 4, OUT_F)


# revision 9
# speedup vs baseline: 1.0373x; 1.0206x over previous
"""LokrLinear TRN2 kernel: out = x @ (W + 2*kron(A@B, O)) + b.

Sharding (8 cores, column-parallel per the hint):
  - Each core owns a 512-column slice of out_features: W[:, c*512:(c+1)*512]
    and the matching 2-column slice of (A@B) (since kron block size = 256).
  - x is replicated, passed pre-transposed as xT (in_f, rows) in bf16.
  - On device: fold W_eff = W_slice + 2*kron((A@B)[:, 2c:2c+2], O) into
    SBUF-resident bf16 weights, then one dense matmul
    out_slice.T = W_eff.T @ xT with fp32 PSUM accumulation; bias is added
    per-partition during PSUM eviction.
  - Host gathers the 8 (512, 16384) outputs, transposes, reshapes.
"""

import numpy as np
import ml_dtypes

P = 128
IN_F = 4096
OUT_F = 4096
ROWS = 4 * 4096            # 16384
N_CORES = 8
COLS = OUT_F // N_CORES    # 512 out_features per core
R = 16                     # LoKr rank
OB = 256                   # O block size (kron block)
JB = COLS // OB            # j-blocks per core = 2
SCALING = 2.0
NS = 512                   # rows per n-slice (one PSUM bank of fp32)
KO = IN_F // P             # 32 k-tiles
MT = COLS // P             # 4 m-tiles

_CACHE = {}


def build_nc(n_slices=ROWS // NS, debug=False):
    """Build the per-core Bass program. Identical on all cores (SPMD);
    core-specific data arrives via the input tensors."""
    import concourse.bass as bass  # noqa: F401
    import concourse.mybir as mybir
    import concourse.tile as tile
    from concourse import bacc

    f32 = mybir.dt.float32
    bf16 = mybir.dt.bfloat16
    rows = n_slices * NS

    nc = bacc.Bacc("TRN2", target_bir_lowering=False, debug=debug)

    xt = nc.dram_tensor("xt", (IN_F, rows), bf16, kind="ExternalInput")
    wk = nc.dram_tensor("wk", (IN_F, COLS), bf16, kind="ExternalInput")
    bias = nc.dram_tensor("bias", (COLS,), f32, kind="ExternalInput")
    a_t = nc.dram_tensor("a_t", (R, R), f32, kind="ExternalInput")
    b_sl = nc.dram_tensor("b_sl", (R, JB), f32, kind="ExternalInput")
    o_mat = nc.dram_tensor("o_mat", (OB, OB), bf16, kind="ExternalInput")
    out = nc.dram_tensor("out", (COLS, rows), f32, kind="ExternalOutput")
    wl_scr = nc.dram_tensor("wl_scr", (R, JB), f32, kind="Internal")

    NPAIR = KO // 2  # 16 k-tile pairs; pair i covers k rows [256i, 256i+256)

    with tile.TileContext(nc) as tc:
        with (
            tc.tile_pool(name="const", bufs=1) as cst,
            tc.tile_pool(name="weff", bufs=NPAIR) as weff_pool,
            tc.tile_pool(name="wraw", bufs=KO // 4) as wraw_pool,
            tc.tile_pool(name="upd", bufs=2) as upd_pool,
            tc.tile_pool(name="xts", bufs=3) as xts_pool,
            tc.tile_pool(name="outp", bufs=4) as out_pool,
            tc.tile_pool(name="ps", bufs=7, space="PSUM") as ps_pool,
            tc.tile_pool(name="wlps", bufs=1, space="PSUM") as wlps_pool,
        ):
            # ---- w_left slice = A @ B[:, 2c:2c+2] (x2 folded into TS) ------
            a_sb = cst.tile((R, R), f32, name="a_sb")
            nc.gpsimd.dma_start(out=a_sb[:], in_=a_t[:, :])
            b_sb = cst.tile((R, JB), f32, name="b_sb")
            nc.gpsimd.dma_start(out=b_sb[:], in_=b_sl[:, :])
            wl_ps = wlps_pool.tile((R, JB), f32, name="wl_ps")
            nc.tensor.matmul(wl_ps[:], a_sb[:], b_sb[:], start=True, stop=True)
            wl_sb = cst.tile((R, JB), f32, name="wl_sb")
            nc.vector.tensor_copy(out=wl_sb[:], in_=wl_ps[:])
            nc.gpsimd.dma_start(out=wl_scr[:, :], in_=wl_sb[:])
            # broadcast the R*JB scalars to every partition: (P, R*JB)
            wl_b = cst.tile((P, R * JB), f32, name="wl_b")
            src = wl_scr[:, :].rearrange("i j -> (i j)")[None, :].to_broadcast(
                [P, R * JB]
            )
            nc.gpsimd.dma_start(out=wl_b[:], in_=src)

            # ---- constants -------------------------------------------------
            o_sb = cst.tile((P, OB // P, OB), bf16, name="o_sb")
            nc.gpsimd.dma_start(
                out=o_sb[:], in_=o_mat[:, :].rearrange("(ah p) b -> p ah b", p=P)
            )
            bias_sb = cst.tile((P, MT), f32, name="bias_sb")
            nc.gpsimd.dma_start(
                out=bias_sb[:], in_=bias[:].rearrange("(m p) -> p m", p=P)
            )

            # ---- W stream: 8 large DMAs, 4 k-tiles (2 pairs) each ----------
            wk_r = wk[:, :].rearrange("(ko p) n -> p ko n", p=P)
            wraws = []
            for q in range(KO // 4):
                wraw = wraw_pool.tile((P, 4, COLS), bf16, name="wraw")
                nc.sync.dma_start(out=wraw[:], in_=wk_r[:, 4 * q : 4 * q + 4, :])
                wraws.append(wraw)

            # ---- fold W_eff = W + 2*kron(w_left, O), one pair at a time ----
            # free-dim layout of pair tiles: (ah, col) with col = jj*OB + b
            weff = []
            for i in range(NPAIR):
                upd = upd_pool.tile((P, 2, COLS), bf16, name="upd")
                for jj in range(JB):
                    nc.vector.tensor_scalar(
                        out=upd[:, :, jj * OB : (jj + 1) * OB],
                        in0=o_sb[:, :, :],
                        scalar1=wl_b[:, i * JB + jj : i * JB + jj + 1],
                        scalar2=SCALING,
                        op0=mybir.AluOpType.mult,
                        op1=mybir.AluOpType.mult,
                    )
                wt = weff_pool.tile((P, 2, COLS), bf16, name="wt")
                nc.vector.tensor_add(
                    out=wt[:],
                    in0=wraws[i // 2][:, (i % 2) * 2 : (i % 2) * 2 + 2, :],
                    in1=upd[:],
                )
                weff.append(wt)

            # ---- main matmul: out.T = W_eff.T @ xT + bias ------------------
            xt_r = xt[:, :].rearrange("(ko p) n -> p ko n", p=P)
            out_r = out[:, :].rearrange("(m p) n -> p m n", p=P)
            for ns in range(n_slices):
                xts = xts_pool.tile((P, KO, NS), bf16, name="xts")
                nc.sync.dma_start(
                    out=xts[:], in_=xt_r[:, :, ns * NS : (ns + 1) * NS]
                )
                for m in range(MT):
                    ps = ps_pool.tile((P, NS), f32, name="ps")
                    for kt in range(KO):
                        nc.tensor.matmul(
                            ps[:],
                            weff[kt // 2][:, kt % 2, m * P : (m + 1) * P],
                            xts[:, kt, :],
                            start=(kt == 0),
                            stop=(kt == KO - 1),
                        )
                    ot = out_pool.tile((P, NS), f32, name="ot")
                    nc.scalar.activation(
                        ot[:],
                        ps[:],
                        mybir.ActivationFunctionType.Identity,
                        bias=bias_sb[:, m : m + 1],
                    )
                    nc.sync.dma_start(
                        out=out_r[:, m, ns * NS : (ns + 1) * NS], in_=ot[:]
                    )

    nc.compile()
    return nc


def _prep_inputs(x, base_kernel, base_bias, lora_A, lora_B, O):
    bf16 = ml_dtypes.bfloat16
    x2d = np.asarray(x, dtype=np.float32).reshape(ROWS, IN_F)
    xt = x2d.T.astype(bf16)  # (IN_F, ROWS) contiguous bf16
    a_t = np.ascontiguousarray(np.asarray(lora_A, np.float32).T)
    o_mat = np.asarray(O, np.float32).astype(bf16)
    in_maps = []
    for c in range(N_CORES):
        in_maps.append(
            {
                "xt": xt,
                "wk": np.asarray(
                    base_kernel[:, c * COLS : (c + 1) * COLS], np.float32
                ).astype(bf16),
                "bias": np.ascontiguousarray(
                    np.asarray(base_bias[c * COLS : (c + 1) * COLS], np.float32)
                ),
                "a_t": a_t,
                "b_sl": np.ascontiguousarray(
                    np.asarray(lora_B[:, c * JB : (c + 1) * JB], np.float32)
                ),
                "o_mat": o_mat,
            }
        )
    return in_maps


def kernel(x, base_kernel, base_bias, lora_A, lora_B, O, _trace=False):
    from concourse.bass_utils import run_bass_kernel_spmd

    if "nc" not in _CACHE:
        _CACHE["nc"] = build_nc()
    nc = _CACHE["nc"]
    in_maps = _prep_inputs(x, base_kernel, base_bias, lora_A, lora_B, O)
    res = run_bass_kernel_spmd(
        nc, in_maps, core_ids=list(range(N_CORES)), trace=_trace
    )
    _CACHE["last_results"] = res
    big = np.concatenate([r["out"] for r in res.results], axis=0)  # (OUT_F, ROWS)
    return np.ascontiguousarray(big.T).reshape(4, ROWS // 4, OUT_F)


# revision 14
# speedup vs baseline: 1.0448x; 1.0072x over previous
"""LokrLinear TRN2 kernel: out = x @ (W + 2*kron(A@B, O)) + b.

Sharding (8 cores, column-parallel per the hint):
  - Each core owns a 512-column slice of out_features: W[:, c*512:(c+1)*512]
    and the matching 2-column slice of (A@B) (since kron block size = 256).
  - x is replicated, passed pre-transposed as xT (in_f, rows) in bf16.
  - On device: fold W_eff = W_slice + 2*kron((A@B)[:, 2c:2c+2], O) into
    SBUF-resident bf16 weights, then one dense matmul
    out_slice.T = W_eff.T @ xT with fp32 PSUM accumulation; bias is added
    per-partition during PSUM eviction.
  - Host gathers the 8 (512, 16384) outputs, transposes, reshapes.
"""

import numpy as np
import ml_dtypes

P = 128
IN_F = 4096
OUT_F = 4096
ROWS = 4 * 4096            # 16384
N_CORES = 8
COLS = OUT_F // N_CORES    # 512 out_features per core
R = 16                     # LoKr rank
OB = 256                   # O block size (kron block)
JB = COLS // OB            # j-blocks per core = 2
SCALING = 2.0
NS = 512                   # rows per n-slice (one PSUM bank of fp32)
KO = IN_F // P             # 32 k-tiles
MT = COLS // P             # 4 m-tiles

_CACHE = {}


def build_nc(n_slices=ROWS // NS, debug=False):
    """Build the per-core Bass program. Identical on all cores (SPMD);
    core-specific data arrives via the input tensors."""
    import concourse.bass as bass  # noqa: F401
    import concourse.mybir as mybir
    import concourse.tile as tile
    from concourse import bacc

    f32 = mybir.dt.float32
    bf16 = mybir.dt.bfloat16
    rows = n_slices * NS

    nc = bacc.Bacc("TRN2", target_bir_lowering=False, debug=debug)

    xt = nc.dram_tensor("xt", (IN_F, rows), bf16, kind="ExternalInput")
    wk = nc.dram_tensor("wk", (IN_F, COLS), bf16, kind="ExternalInput")
    bias = nc.dram_tensor("bias", (COLS,), f32, kind="ExternalInput")
    a_t = nc.dram_tensor("a_t", (R, R), f32, kind="ExternalInput")
    b_sl = nc.dram_tensor("b_sl", (R, JB), f32, kind="ExternalInput")
    o_mat = nc.dram_tensor("o_mat", (OB, OB), bf16, kind="ExternalInput")
    out = nc.dram_tensor("out", (COLS, rows), f32, kind="ExternalOutput")
    wl_scr = nc.dram_tensor("wl_scr", (R, JB), f32, kind="Internal")

    NPAIR = KO // 2  # 16 k-tile pairs; pair i covers k rows [256i, 256i+256)

    with tile.TileContext(nc) as tc:
        with (
            tc.tile_pool(name="const", bufs=1) as cst,
            tc.tile_pool(name="weff", bufs=NPAIR) as weff_pool,
            tc.tile_pool(name="wraw", bufs=KO // 4) as wraw_pool,
            tc.tile_pool(name="upd", bufs=2) as upd_pool,
            tc.tile_pool(name="xts", bufs=12) as xts_pool,
            tc.tile_pool(name="outp", bufs=4) as out_pool,
            tc.tile_pool(name="ps", bufs=7, space="PSUM") as ps_pool,
            tc.tile_pool(name="wlps", bufs=1, space="PSUM") as wlps_pool,
        ):
            # ---- w_left slice = A @ B[:, 2c:2c+2] (x2 folded into TS) ------
            a_sb = cst.tile((R, R), f32, name="a_sb")
            nc.sync.dma_start(out=a_sb[:], in_=a_t[:, :])
            b_sb = cst.tile((R, JB), f32, name="b_sb")
            nc.sync.dma_start(out=b_sb[:], in_=b_sl[:, :])
            wl_ps = wlps_pool.tile((R, JB), f32, name="wl_ps")
            nc.tensor.matmul(wl_ps[:], a_sb[:], b_sb[:], start=True, stop=True)
            wl_sb = cst.tile((R, JB), f32, name="wl_sb")
            nc.vector.tensor_copy(out=wl_sb[:], in_=wl_ps[:])
            nc.gpsimd.dma_start(out=wl_scr[:, :], in_=wl_sb[:])
            # broadcast the R*JB scalars to every partition: (P, R*JB)
            wl_b = cst.tile((P, R * JB), f32, name="wl_b")
            src = wl_scr[:, :].rearrange("i j -> (i j)")[None, :].to_broadcast(
                [P, R * JB]
            )
            nc.gpsimd.dma_start(out=wl_b[:], in_=src)

            # ---- constants -------------------------------------------------
            o_sb = cst.tile((P, OB // P, OB), bf16, name="o_sb")
            nc.gpsimd.dma_start(
                out=o_sb[:], in_=o_mat[:, :].rearrange("(ah p) b -> p ah b", p=P)
            )
            bias_sb = cst.tile((P, MT), f32, name="bias_sb")
            nc.gpsimd.dma_start(
                out=bias_sb[:], in_=bias[:].rearrange("(m p) -> p m", p=P)
            )

            # ---- PE warmup: junk matmuls so HAM un-throttles before the
            # real stream starts (one accumulation group, overwritten) ------
            warm_ps = ps_pool.tile((P, NS), f32, name="ps")
            for w in range(16):
                nc.tensor.matmul(
                    warm_ps[:],
                    o_sb[:, 0, 0:P],
                    o_sb[:, :, :],
                    start=(w == 0),
                    stop=(w == 15),
                )

            # ---- W stream: 8 large DMAs, 4 k-tiles (2 pairs) each ----------
            wk_r = wk[:, :].rearrange("(ko p) n -> p ko n", p=P)
            wraws = []
            for q in range(KO // 4):
                wraw = wraw_pool.tile((P, 4, COLS), bf16, name="wraw")
                nc.sync.dma_start(out=wraw[:], in_=wk_r[:, 4 * q : 4 * q + 4, :])
                wraws.append(wraw)

            # ---- fold W_eff = W + 2*kron(w_left, O), one pair at a time ----
            # free-dim layout of pair tiles: (ah, col) with col = jj*OB + b
            weff = []
            for i in range(NPAIR):
                upd = upd_pool.tile((P, 2, COLS), bf16, name="upd")
                for jj in range(JB):
                    nc.vector.tensor_scalar(
                        out=upd[:, :, jj * OB : (jj + 1) * OB],
                        in0=o_sb[:, :, :],
                        scalar1=wl_b[:, i * JB + jj : i * JB + jj + 1],
                        scalar2=SCALING,
                        op0=mybir.AluOpType.mult,
                        op1=mybir.AluOpType.mult,
                    )
                wt = weff_pool.tile((P, 2, COLS), bf16, name="wt")
                nc.vector.tensor_add(
                    out=wt[:],
                    in0=wraws[i // 2][:, (i % 2) * 2 : (i % 2) * 2 + 2, :],
                    in1=upd[:],
                )
                weff.append(wt)

            # ---- main matmul: out.T = W_eff.T @ xT + bias ------------------
            xt_r = xt[:, :].rearrange("(ko p) n -> p ko n", p=P)
            out_r = out[:, :].rearrange("(m p) n -> p m n", p=P)
            for ns in range(n_slices):
                xq = []
                for q in range(4):
                    xt_tile = xts_pool.tile((P, KO // 4, NS), bf16, name="xt_tile")
                    nc.sync.dma_start(
                        out=xt_tile[:],
                        in_=xt_r[:, 8 * q : 8 * q + 8, ns * NS : (ns + 1) * NS],
                    )
                    xq.append(xt_tile)
                for m in range(MT):
                    ps = ps_pool.tile((P, NS), f32, name="ps")
                    for kt in range(KO):
                        nc.tensor.matmul(
                            ps[:],
                            weff[kt // 2][:, kt % 2, m * P : (m + 1) * P],
                            xq[kt // 8][:, kt % 8, :],
                            start=(kt == 0),
                            stop=(kt == KO - 1),
                        )
                    ot = out_pool.tile((P, NS), f32, name="ot")
                    nc.scalar.activation(
                        ot[:],
                        ps[:],
                        mybir.ActivationFunctionType.Identity,
                        bias=bias_sb[:, m : m + 1],
                    )
                    nc.sync.dma_start(
                        out=out_r[:, m, ns * NS : (ns + 1) * NS], in_=ot[:]
                    )

    nc.compile()
    return nc


def _prep_inputs(x, base_kernel, base_bias, lora_A, lora_B, O):
    bf16 = ml_dtypes.bfloat16
    x2d = np.asarray(x, dtype=np.float32).reshape(ROWS, IN_F)
    xt = x2d.T.astype(bf16)  # (IN_F, ROWS) contiguous bf16
    a_t = np.ascontiguousarray(np.asarray(lora_A, np.float32).T)
    o_mat = np.asarray(O, np.float32).astype(bf16)
    in_maps = []
    for c in range(N_CORES):
        in_maps.append(
            {
                "xt": xt,
                "wk": np.asarray(
                    base_kernel[:, c * COLS : (c + 1) * COLS], np.float32
                ).astype(bf16),
                "bias": np.ascontiguousarray(
                    np.asarray(base_bias[c * COLS : (c + 1) * COLS], np.float32)
                ),
                "a_t": a_t,
                "b_sl": np.ascontiguousarray(
                    np.asarray(lora_B[:, c * JB : (c + 1) * JB], np.float32)
                ),
                "o_mat": o_mat,
            }
        )
    return in_maps


def kernel(x, base_kernel, base_bias, lora_A, lora_B, O, _trace=False):
    from concourse.bass_utils import run_bass_kernel_spmd

    if "nc" not in _CACHE:
        _CACHE["nc"] = build_nc()
    nc = _CACHE["nc"]
    in_maps = _prep_inputs(x, base_kernel, base_bias, lora_A, lora_B, O)
    res = run_bass_kernel_spmd(
        nc, in_maps, core_ids=list(range(N_CORES)), trace=_trace
    )
    _CACHE["last_results"] = res
    big = np.concatenate([r["out"] for r in res.results], axis=0)  # (OUT_F, ROWS)
    return np.ascontiguousarray(big.T).reshape(4, ROWS // 4, OUT_F)
